# revision 1
# baseline (speedup 1.0000x reference)
"""EquivariantDense kernel for Trainium2 (8 NeuronCores, data-parallel over batch).

Math: with K = 4096, N = 4K, shift = K, the reference computes
    out[b, i*O4 + o] = sum_j sum_k w_{j+1}[b, o, k] * x[b, ((i+j)%4)*K + k]
i.e. per batch, 4 weight matrices (1024, 4096) each hit the 4 chunks of x.

Device mapping (per core = one batch):
  - PE matmul out[m,n] = sum_p lhsT[p,m] * rhs[p,n] contracts over partitions,
    so weights are staged (on host) transposed to (k, o) layout.
  - stationary lhsT = x-chunk tile (128 k-part, 4 roll-columns) -> tiny LDWEIGHTS
  - moving rhs = W^T tile (128 k-part, 512 o) fp32
  - accumulate all 128 k-blocks (4 j * 32 kb) into PSUM (4, 512) x 2 o-halves
  - weights stream HBM->SBUF as 32 contiguous 2 MiB DMAs (memory-bound regime,
    ~64 MiB/core at ~358 GB/s HBM/core roofline)
"""

import numpy as np

import concourse.mybir as mybir
import concourse.tile as tile
from concourse import bacc, bass_utils

B = 8
O4 = 1024
K = 4096
N = 4 * K  # 16384
NBLK = N // 128  # 128 global k-blocks of 128
KB2 = 4  # k-blocks per DMA tile
NT = NBLK // KB2  # 32 DMA tiles, 2 MiB each

_nc_cache = None


def _build_program(repeat=1):
    # repeat>1 builds the same body repeated back-to-back; used only for
    # timing measurements (dispatch-overhead-free per-iteration estimates)
    nc = bacc.Bacc()
    f32 = mybir.dt.float32
    xs_d = nc.dram_tensor("xstat", [128, NBLK * 4], f32, kind="ExternalInput")
    wt_d = nc.dram_tensor("wt", [NT, 128, KB2 * O4], f32, kind="ExternalInput")
    out_d = nc.dram_tensor("out", [4, O4], f32, kind="ExternalOutput")

    with tile.TileContext(nc) as tc:
        with (
            tc.tile_pool(name="xp", bufs=1) as xp,
            tc.tile_pool(name="wp", bufs=4) as wp,
            tc.tile_pool(name="pp", bufs=2, space="PSUM") as pp,
            tc.tile_pool(name="op", bufs=2) as op,
        ):
            xs = xp.tile([128, NBLK * 4], f32)
            # SWDGE: keeps the SP HWDGE ring free for the weight stream.
            # (Loading xs via the ACT HWDGE ring instead correlated with
            # NRT_EXEC_UNIT_UNRECOVERABLE crashes under concurrent
            # dual-ring DMA; SWDGE here has been stable across many runs.)
            nc.gpsimd.dma_start(xs[:], xs_d[:])
            for _rep in range(repeat):
                ps0 = pp.tile([4, 512], f32, tag="ps0")
                ps1 = pp.tile([4, 512], f32, tag="ps1")
                # Read tiles highest-address-first: reverse of the input
                # upload order, so if the memory system keeps recently
                # written lines warm, the single cold pass hits them first.
                # Order is otherwise irrelevant (PSUM accumulation commutes).
                for tidx, t in enumerate(reversed(range(NT))):
                    w_tile = wp.tile([128, KB2 * O4], f32, tag="w")
                    if tidx < NT - 1:
                        nc.sync.dma_start(w_tile[:], wt_d[t])
                    else:
                        # split the last-issued tile per k-block so the final
                        # matmuls chase the stream and the tail stays short;
                        # the final k-block splits again per o-half so the
                        # very last matmul waits on only 256 KiB
                        for kk in range(KB2 - 1):
                            nc.sync.dma_start(
                                w_tile[:, kk * O4 : (kk + 1) * O4],
                                wt_d[t, :, kk * O4 : (kk + 1) * O4],
                            )
                        kk = KB2 - 1
                        nc.sync.dma_start(
                            w_tile[:, kk * O4 : kk * O4 + 512],
                            wt_d[t, :, kk * O4 : kk * O4 + 512],
                        )
                        nc.sync.dma_start(
                            w_tile[:, kk * O4 + 512 : (kk + 1) * O4],
                            wt_d[t, :, kk * O4 + 512 : (kk + 1) * O4],
                        )
                    for kb2 in range(KB2):
                        g = t * KB2 + kb2
                        lhsT = xs[:, g * 4 : (g + 1) * 4]
                        first = tidx == 0 and kb2 == 0
                        last = tidx == NT - 1 and kb2 == KB2 - 1
                        nc.tensor.matmul(
                            ps0[:],
                            lhsT,
                            w_tile[:, kb2 * O4 : kb2 * O4 + 512],
                            start=first,
                            stop=last,
                        )
                        nc.tensor.matmul(
                            ps1[:],
                            lhsT,
                            w_tile[:, kb2 * O4 + 512 : (kb2 + 1) * O4],
                            start=first,
                            stop=last,
                        )
                ot = op.tile([4, O4], f32, tag="ot")
                nc.vector.tensor_copy(ot[:, 0:512], ps0[:])
                nc.scalar.copy(ot[:, 512:O4], ps1[:])
                nc.sync.dma_start(out_d[:], ot[:])
    nc.compile()
    return nc


def _get_program():
    global _nc_cache
    if _nc_cache is None:
        _nc_cache = _build_program()
    return _nc_cache


def prepare_inputs(x, w1, w2, w3, w4):
    """Host-side marshalling: shard over batch, transpose W to (k, o) tiles."""
    x = np.ascontiguousarray(np.asarray(x), dtype=np.float32)
    # Weight staging: W[b, j, o, k] -> Wh[b, t, p, kb2*O4 + o]
    # where k = (t*4 + kb2)*128 + p and j = (t*4 + kb2) // 32.
    W = np.stack(
        [np.asarray(w, dtype=np.float32) for w in (w1, w2, w3, w4)], axis=1
    )  # (B, 4, O4, K)
    W6 = W.reshape(B, 4, O4, 8, KB2, 128)  # k = tq*512 + kb2*128 + p
    Wh = np.ascontiguousarray(W6.transpose(0, 1, 3, 5, 4, 2)).reshape(
        B, NT, 128, KB2 * O4
    )

    # x staging: xs[b, p, g*4 + c] = x[b, ((c + g//32) % 4)*K + (g%32)*128 + p]
    cols = np.arange(NBLK * 4)
    g = cols // 4
    c = cols % 4
    j = g // 32
    kb = g % 32
    src_base = ((c + j) % 4) * K + kb * 128  # (512,)
    xs = x[:, src_base[None, :] + np.arange(128)[:, None]]  # (B, 128, 512)
    xs = np.ascontiguousarray(xs, dtype=np.float32)
    return xs, Wh


def run(x, w1, w2, w3, w4, trace=False, **kwargs):
    xs, Wh = prepare_inputs(x, w1, w2, w3, w4)
    nc = _get_program()
    in_maps = [{"xstat": xs[b], "wt": Wh[b]} for b in range(B)]
    res = bass_utils.run_bass_kernel_spmd(
        nc, in_maps, list(range(B)), trace=trace, **kwargs
    )
    out = np.stack(
        [res.results[b]["out"].reshape(4 * O4) for b in range(B)]
    ).astype(np.float32)
    return out, res


def kernel(x, w1, w2, w3, w4):
    out, _ = run(x, w1, w2, w3, w4)
    return out

